# revision 17
# baseline (speedup 1.0000x reference)
"""SAGAN-style attention block (nn_AttentionBlock) on 8 Trainium2 NeuronCores.

Problem (per sample): x [C=64, N=4096] (N = 64x64 spatial),
  f = Wf x + bf   [8, N]
  g = Wg x + bg   [8, N]
  h = Wh x + bh   [64, N]
  s = f^T g       [N, N]
  beta = softmax(s, axis=1)  (over j)
  o[c, i] = sum_j h[c, j] beta[i, j]
  out = x + o

Sharding: pure data parallel over batch B=8 -> one sample per core.

Per-core algorithm (everything [channels-on-partitions]):
  - One consolidated input DMA: x_aug = [x; ones] [65, 4096] plus the
    (transposed, bias-augmented) weights.
  - Projections on the PE in plain fp32:
      fg-chunk  = [WfT|WgT; bf|bg]^T @ x_aug_chunk        -> [16, 512]
      hT-tile   = x_aug_tile^T @ [WhT; bh]                -> [128, 64]
    evacuated by DVE copies into float32r SBUF tiles (f, g, hT_aug),
    where hT_aug gets a 65th all-ones column (computes the softmax
    denominator for free inside the o-matmul).
  - Main loop over 8 i-chunks (512 queries) x 32 j-tiles (128 keys):
      sT-tile [j=128, i=512] = g_tile^T @ f_chunk      (f32r matmul, K=8)
      e = exp(sT)  on ACT, PSUM->SBUF, f32r            (no max-subtraction:
          measured scores are in [-31, 30], fp32 exp is exact enough)
      o_aug [65, i=512] += hT_aug_tile^T @ e           (f32r matmul, K=128)
    j-tiles processed in groups of 3 sharing a 3-bank PSUM macro-tile so
    each ACT exp instruction covers 1536 elements/partition.
  - Finalize per i-chunk: row 64 of o_aug is the softmax denominator.
      oc   = ACT copy of o_aug (PSUM->SBUF)  [the only PSUM reader]
      r    = DVE reciprocal of oc[64]        [1, 512]
      rb   = PE broadcast matmul ones[1,64]^T @ r -> [64, 512] PSUM
      out  = oc[0:64] * rb + x   (DVE)
  - Two output DMAs of [64, 2048].
"""
import sys

sys.path.insert(0, "/opt/trn_rl_repo")

import numpy as np
from contextlib import ExitStack

import concourse.bass as bass  # noqa: F401  (bacc subclasses Bass)
import concourse.tile as tile
from concourse import bacc, mybir
from concourse.bass_utils import run_bass_kernel_spmd

F32 = mybir.dt.float32
F32R = mybir.dt.float32r
BF16 = mybir.dt.bfloat16
MMDT = BF16   # dtype for the big s/o matmul operands

B, C, H, W = 8, 64, 64, 64
N = H * W          # 4096
C8 = 8             # f/g channels
NCORES = 8
ICHUNK = 512       # i (query) tile width; one PSUM bank
NI = N // ICHUNK   # 8
JT = 128           # j (key) tile = PSUM partitions
NJ = N // JT       # 32
GROUP = 3          # j-tiles per PSUM macro tile / per exp instruction

_CACHE = {}


def _build_nc():
    nc = bacc.Bacc("TRN2", target_bir_lowering=False, debug=False,
                   num_devices=NCORES)
    # input layout: [65, 4096 + 16 + 64]
    #   [:, 0:4096]      x_aug (row 64 = ones)
    #   [:, 4096:4112]   wfg_aug = [[Wf^T, Wg^T]; [bf, bg]]
    #   [:, 4112:4176]   wh_aug  = [Wh^T; bh]
    inp = nc.dram_tensor("inp", [C + 1, N + 16 + C], F32,
                         kind="ExternalInput").ap()
    out = nc.dram_tensor("out", [C, N], F32, kind="ExternalOutput").ap()

    with tile.TileContext(nc) as tc:
        with ExitStack() as ctx:
            sb = ctx.enter_context(tc.tile_pool(name="sb", bufs=1))
            epool = ctx.enter_context(tc.tile_pool(name="ep", bufs=6))
            fin = ctx.enter_context(tc.tile_pool(name="fin", bufs=8))
            psA = ctx.enter_context(tc.tile_pool(name="psA", bufs=1, space="PSUM"))
            psB = ctx.enter_context(tc.tile_pool(name="psB", bufs=1, space="PSUM"))
            psO1 = ctx.enter_context(tc.tile_pool(name="psO1", bufs=1, space="PSUM"))
            psO2 = ctx.enter_context(tc.tile_pool(name="psO2", bufs=1, space="PSUM"))
            dram = ctx.enter_context(tc.tile_pool(name="dram", bufs=8, space="DRAM"))

            # ---- constants (early, low DVE ticks) ----
            ones_f = sb.tile([128, 1], F32)
            nc.vector.memset(ones_f[:], 1.0)
            ones_b = sb.tile([1, ICHUNK], MMDT)
            nc.vector.tensor_copy(ones_b[:], ones_f[0:1, 0:1].to_broadcast((1, ICHUNK)))

            # ---- input: weights first, then x in quarters (overlap) ----
            tin = sb.tile([C + 1, N + 16 + C], F32)
            nc.sync.dma_start(tin[:, N:N + 16 + C], inp[:, N:N + 16 + C])
            for c in range(4):
                nc.sync.dma_start(tin[:, c * 1024:(c + 1) * 1024],
                                  inp[:, c * 1024:(c + 1) * 1024])
            x_aug = tin[:, 0:N]
            wfg = tin[:, N:N + 16]
            wh = tin[:, N + 16:N + 16 + C]

            pools4 = [(psA, "m"), (psB, "m"), (psO1, "po1"), (psO2, "po2")]

            # ---- PE warmup: ~9us of back-to-back dummy matmuls while the
            # input DMA runs, so the HAM clock gate opens (1.2 -> 2.4 GHz)
            # before the real work starts ----
            warm_ps = psO1.tile([JT, ICHUNK], F32, tag="po1", name="warmps")
            for _ in range(20):
                nc.tensor.matmul(warm_ps[:], ones_b[0:1, 0:JT], ones_b[:],
                                 start=True, stop=True)

            # ---- projections: f and g first (the main loop needs them
            # immediately); replicate to partition offsets 32/64 via fast
            # bf16 DVE copies for the 3x row-packed s-matmuls ----
            f_sb = sb.tile([72, N], MMDT)
            g_sb = sb.tile([72, N], MMDT)
            for c in range(NI):
                xc = x_aug[:, c * ICHUNK:(c + 1) * ICHUNK]
                cs = slice(c * ICHUNK, (c + 1) * ICHUNK)
                _pl, _tg = pools4[c % 4]
                ppf = _pl.tile([C8, ICHUNK], F32, tag=_tg, name=f"ppf{c}")
                nc.tensor.matmul(ppf[:], wfg[:, 0:C8], xc, start=True, stop=True)
                nc.scalar.copy(f_sb[0:C8, cs], ppf[:])
                _pl, _tg = pools4[(c + 2) % 4]
                ppg = _pl.tile([C8, ICHUNK], F32, tag=_tg, name=f"ppg{c}")
                nc.tensor.matmul(ppg[:], wfg[:, C8:16], xc, start=True, stop=True)
                nc.vector.tensor_copy(g_sb[0:C8, cs], ppg[:])
            for r in (32, 64):
                nc.vector.tensor_copy(f_sb[r:r + C8, :], f_sb[0:C8, :])
                nc.vector.tensor_copy(g_sb[r:r + C8, :], g_sb[0:C8, :])

            # bf16 copies of x_aug / wh for the hT projections (cheap
            # LDWEIGHTS; h-path precision loss is ~0.4%, acceptable)
            xb = sb.tile([C + 1, N], MMDT)
            for c in range(4):
                if c % 2 == 0:
                    nc.vector.tensor_copy(xb[:, c * 1024:(c + 1) * 1024],
                                          x_aug[:, c * 1024:(c + 1) * 1024])
                else:
                    nc.scalar.copy(xb[:, c * 1024:(c + 1) * 1024],
                                   x_aug[:, c * 1024:(c + 1) * 1024])
            whb = sb.tile([C + 1, C], MMDT)
            nc.vector.tensor_copy(whb[:], wh)

            # ---- projections: hT (4 j-tiles per PSUM bank, one evac each) ----
            hT = sb.tile([JT, NJ, C + 1], MMDT)
            for t4 in range(NJ // 4):
                _pl, _tg = pools4[t4 % 4]
                pp = _pl.tile([JT, 4 * C], F32, tag=_tg, name=f"pph{t4}")
                for u in range(4):
                    t = 4 * t4 + u
                    nc.tensor.matmul(pp[:, u * C:(u + 1) * C],
                                     xb[:, t * JT:(t + 1) * JT], whb[:],
                                     start=True, stop=True)
                nc.vector.tensor_copy(
                    hT[:, 4 * t4:4 * t4 + 4, 0:C],
                    pp[:].rearrange("p (a b) -> p a b", a=4))
            nc.vector.tensor_copy(hT[:, :, C:C + 1],
                                  ones_f[:].to_broadcast((JT, NJ, 1)))

            # ---- main attention loop ----
            res = sb.tile([C, N], F32)    # final output staging
            groups = []
            j0 = 0
            while j0 < NJ:
                groups.append((j0, min(GROUP, NJ - j0)))
                j0 += GROUP

            def emit_o(po, e, j0, glen):
                for k in range(glen):
                    j = j0 + k
                    nc.tensor.matmul(
                        po[j % 2][:], hT[:, j, :],
                        e[:, k * ICHUNK:(k + 1) * ICHUNK],
                        start=(j < 2), stop=(j >= NJ - 2))

            def emit_fin(po, q):
                # finalize, all off the PE: merge halves, recip, DMA-bounce
                # broadcast of 1/den, divide, residual
                qs = slice(q * ICHUNK, (q + 1) * ICHUNK)
                oc = fin.tile([C + 1, ICHUNK], F32, tag="oc",
                              name=f"oc{q}")
                nc.vector.tensor_copy(oc[:], po[0][:])
                nc.vector.tensor_add(oc[:], oc[:], po[1][:])
                r = fin.tile([1, ICHUNK], F32, tag="r", name=f"r{q}")
                nc.vector.reciprocal(r[:], oc[C:C + 1, :])
                scr = dram.tile([1, ICHUNK], F32, tag="scr", name=f"scr{q}")
                nc.sync.dma_start(scr[:], r[:])
                rb = fin.tile([C, ICHUNK], F32, tag="rb", name=f"rb{q}")
                nc.sync.dma_start(rb[:], scr[:].to_broadcast((C, ICHUNK)))
                nc.vector.tensor_mul(res[:, qs], oc[0:C, :], rb[:])
                nc.vector.tensor_add(res[:, qs], res[:, qs], tin[0:C, qs])
                nc.sync.dma_start(out[:, qs], res[:, qs])

            # Software-pipelined emission: the o-matmuls of group m are
            # emitted AFTER the s-trio of group m+1, so the in-order PE
            # queue can run that s-trio while ACT is still computing
            # exp(m) (o-matmuls of m must wait for it).
            gidx = 0          # global group counter for A/B alternation
            pend_o = None     # (po, e, j0, glen) of the previous group
            pend_fin = None   # (po, q) once a chunk's last o is emitted
            po = None
            for q in range(NI):
                qs = slice(q * ICHUNK, (q + 1) * ICHUNK)
                po1 = psO1.tile([C + 1, ICHUNK], F32, tag="po1", name=f"po1_{q}")
                po2 = psO2.tile([C + 1, ICHUNK], F32, tag="po2", name=f"po2_{q}")
                po = [po1, po2]
                for gi, (j0, glen) in enumerate(groups):
                    pool = psA if gidx % 2 == 0 else psB
                    gidx += 1
                    pm = pool.tile([JT, GROUP * ICHUNK], F32, tag="m")
                    for k in range(glen):
                        j = j0 + k
                        nc.tensor.matmul(
                            pm[:, k * ICHUNK:(k + 1) * ICHUNK],
                            g_sb[32 * k:32 * k + C8, j * JT:(j + 1) * JT],
                            f_sb[32 * k:32 * k + C8, qs],
                            start=True, stop=True)
                    e = epool.tile([JT, GROUP * ICHUNK], MMDT, tag="e")
                    nc.scalar.activation(e[:, 0:glen * ICHUNK],
                                         pm[:, 0:glen * ICHUNK],
                                         mybir.ActivationFunctionType.Exp)
                    if pend_o is not None:
                        emit_o(*pend_o)
                    if pend_fin is not None:
                        emit_fin(*pend_fin)
                        pend_fin = None
                    pend_o = (po, e, j0, glen)
                pend_fin = (po, q)
            emit_o(*pend_o)
            emit_fin(*pend_fin)
    nc.compile()
    return nc


def _marshal(x_b, Wf, bf, Wg, bg, Wh, bh):
    """Build the per-core [65, 4176] input block."""
    xa = np.empty((C + 1, N + 16 + C), dtype=np.float32)
    xa[0:C, 0:N] = x_b.reshape(C, N)
    xa[C, 0:N] = 1.0
    xa[0:C, N:N + C8] = Wf.T
    xa[C, N:N + C8] = bf
    xa[0:C, N + C8:N + 16] = Wg.T
    xa[C, N + C8:N + 16] = bg
    xa[0:C, N + 16:N + 16 + C] = Wh.T
    xa[C, N + 16:N + 16 + C] = bh
    return xa


LAST_RESULTS = None


def kernel(x, Wf, bf, Wg, bg, Wh, bh):
    global LAST_RESULTS
    x = np.asarray(x, dtype=np.float32)
    Wf = np.asarray(Wf, dtype=np.float32)
    bf = np.asarray(bf, dtype=np.float32)
    Wg = np.asarray(Wg, dtype=np.float32)
    bg = np.asarray(bg, dtype=np.float32)
    Wh = np.asarray(Wh, dtype=np.float32)
    bh = np.asarray(bh, dtype=np.float32)

    if "nc" not in _CACHE:
        _CACHE["nc"] = _build_nc()
    nc = _CACHE["nc"]

    in_maps = [{"inp": _marshal(x[b], Wf, bf, Wg, bg, Wh, bh)}
               for b in range(NCORES)]
    res = run_bass_kernel_spmd(nc, in_maps, list(range(NCORES)))
    LAST_RESULTS = res
    out = np.stack([res.results[b]["out"] for b in range(NCORES)], axis=0)
    return out.reshape(B, C, H, W).astype(np.float32)


# revision 19
# speedup vs baseline: 1.0480x; 1.0480x over previous
"""SAGAN-style attention block (nn_AttentionBlock) on 8 Trainium2 NeuronCores.

Problem (per sample): x [C=64, N=4096] (N = 64x64 spatial),
  f = Wf x + bf   [8, N]
  g = Wg x + bg   [8, N]
  h = Wh x + bh   [64, N]
  s = f^T g       [N, N]
  beta = softmax(s, axis=1)  (over j)
  o[c, i] = sum_j h[c, j] beta[i, j]
  out = x + o

Sharding: pure data parallel over batch B=8 -> one sample per core.

Per-core algorithm (everything [channels-on-partitions]):
  - One consolidated input DMA: x_aug = [x; ones] [65, 4096] plus the
    (transposed, bias-augmented) weights.
  - Projections on the PE in plain fp32:
      fg-chunk  = [WfT|WgT; bf|bg]^T @ x_aug_chunk        -> [16, 512]
      hT-tile   = x_aug_tile^T @ [WhT; bh]                -> [128, 64]
    evacuated by DVE copies into float32r SBUF tiles (f, g, hT_aug),
    where hT_aug gets a 65th all-ones column (computes the softmax
    denominator for free inside the o-matmul).
  - Main loop over 8 i-chunks (512 queries) x 32 j-tiles (128 keys):
      sT-tile [j=128, i=512] = g_tile^T @ f_chunk      (f32r matmul, K=8)
      e = exp(sT)  on ACT, PSUM->SBUF, f32r            (no max-subtraction:
          measured scores are in [-31, 30], fp32 exp is exact enough)
      o_aug [65, i=512] += hT_aug_tile^T @ e           (f32r matmul, K=128)
    j-tiles processed in groups of 3 sharing a 3-bank PSUM macro-tile so
    each ACT exp instruction covers 1536 elements/partition.
  - Finalize per i-chunk: row 64 of o_aug is the softmax denominator.
      oc   = ACT copy of o_aug (PSUM->SBUF)  [the only PSUM reader]
      r    = DVE reciprocal of oc[64]        [1, 512]
      rb   = PE broadcast matmul ones[1,64]^T @ r -> [64, 512] PSUM
      out  = oc[0:64] * rb + x   (DVE)
  - Two output DMAs of [64, 2048].
"""
import sys

sys.path.insert(0, "/opt/trn_rl_repo")

import numpy as np
from contextlib import ExitStack

import concourse.bass as bass  # noqa: F401  (bacc subclasses Bass)
import concourse.tile as tile
from concourse import bacc, mybir
from concourse.bass_utils import run_bass_kernel_spmd

F32 = mybir.dt.float32
F32R = mybir.dt.float32r
BF16 = mybir.dt.bfloat16
MMDT = BF16   # dtype for the big s/o matmul operands

B, C, H, W = 8, 64, 64, 64
N = H * W          # 4096
C8 = 8             # f/g channels
NCORES = 8
ICHUNK = 512       # i (query) tile width; one PSUM bank
NI = N // ICHUNK   # 8
JT = 128           # j (key) tile = PSUM partitions
NJ = N // JT       # 32
GROUP = 3          # j-tiles per PSUM macro tile / per exp instruction

_CACHE = {}


def _build_nc():
    nc = bacc.Bacc("TRN2", target_bir_lowering=False, debug=False,
                   num_devices=NCORES)
    # input layout: [65, 4096 + 16 + 64]
    #   [:, 0:4096]      x_aug (row 64 = ones)
    #   [:, 4096:4112]   wfg_aug = [[Wf^T, Wg^T]; [bf, bg]]
    #   [:, 4112:4176]   wh_aug  = [Wh^T; bh]
    inp = nc.dram_tensor("inp", [C + 1, N + 16 + C], F32,
                         kind="ExternalInput").ap()
    out = nc.dram_tensor("out", [C, N], F32, kind="ExternalOutput").ap()

    with tile.TileContext(nc) as tc:
        with ExitStack() as ctx:
            sb = ctx.enter_context(tc.tile_pool(name="sb", bufs=1))
            epool = ctx.enter_context(tc.tile_pool(name="ep", bufs=6))
            fin = ctx.enter_context(tc.tile_pool(name="fin", bufs=8))
            psA = ctx.enter_context(tc.tile_pool(name="psA", bufs=1, space="PSUM"))
            psB = ctx.enter_context(tc.tile_pool(name="psB", bufs=1, space="PSUM"))
            psO1 = ctx.enter_context(tc.tile_pool(name="psO1", bufs=1, space="PSUM"))
            psO2 = ctx.enter_context(tc.tile_pool(name="psO2", bufs=1, space="PSUM"))
            dram = ctx.enter_context(tc.tile_pool(name="dram", bufs=8, space="DRAM"))

            # ---- constants (early, low DVE ticks) ----
            ones_f = sb.tile([128, 1], F32)
            nc.vector.memset(ones_f[:], 1.0)

            # ---- input: weights first, then x in quarters (overlap) ----
            tin = sb.tile([C + 1, N + 16 + C], F32)
            nc.sync.dma_start(tin[:, N:N + 16 + C], inp[:, N:N + 16 + C])
            for c in range(4):
                nc.sync.dma_start(tin[:, c * 1024:(c + 1) * 1024],
                                  inp[:, c * 1024:(c + 1) * 1024])
            x_aug = tin[:, 0:N]
            wfg = tin[:, N:N + 16]
            wh = tin[:, N + 16:N + 16 + C]

            pools4 = [(psA, "m"), (psB, "m"), (psO1, "po1"), (psO2, "po2")]

            # ---- projections: f and g first (the main loop needs them
            # immediately); replicate to partition offsets 32/64 via fast
            # bf16 DVE copies for the 3x row-packed s-matmuls ----
            f_sb = sb.tile([72, N], MMDT)
            g_sb = sb.tile([72, N], MMDT)
            for c in range(NI):
                xc = x_aug[:, c * ICHUNK:(c + 1) * ICHUNK]
                cs = slice(c * ICHUNK, (c + 1) * ICHUNK)
                _pl, _tg = pools4[c % 4]
                ppf = _pl.tile([C8, ICHUNK], F32, tag=_tg, name=f"ppf{c}")
                nc.tensor.matmul(ppf[:], wfg[:, 0:C8], xc, start=True, stop=True)
                nc.vector.tensor_copy(f_sb[0:C8, cs], ppf[:])
                _pl, _tg = pools4[(c + 2) % 4]
                ppg = _pl.tile([C8, ICHUNK], F32, tag=_tg, name=f"ppg{c}")
                nc.tensor.matmul(ppg[:], wfg[:, C8:16], xc, start=True, stop=True)
                nc.vector.tensor_copy(g_sb[0:C8, cs], ppg[:])
            for r in (32, 64):
                nc.vector.tensor_copy(f_sb[r:r + C8, :], f_sb[0:C8, :])
                nc.vector.tensor_copy(g_sb[r:r + C8, :], g_sb[0:C8, :])

            # bf16 copies of x_aug / wh for the hT projections (cheap
            # LDWEIGHTS; h-path precision loss is ~0.4%, acceptable)
            xb = sb.tile([C + 1, N], MMDT)
            for c in range(4):
                nc.vector.tensor_copy(xb[:, c * 1024:(c + 1) * 1024],
                                      x_aug[:, c * 1024:(c + 1) * 1024])
            whb = sb.tile([C + 1, C], MMDT)
            nc.vector.tensor_copy(whb[:], wh)

            # ---- projections: hT (4 j-tiles per PSUM bank, one evac each) ----
            hT = sb.tile([JT, NJ, C + 1], MMDT)
            for t4 in range(NJ // 4):
                _pl, _tg = pools4[t4 % 4]
                pp = _pl.tile([JT, 4 * C], F32, tag=_tg, name=f"pph{t4}")
                for u in range(4):
                    t = 4 * t4 + u
                    nc.tensor.matmul(pp[:, u * C:(u + 1) * C],
                                     xb[:, t * JT:(t + 1) * JT], whb[:],
                                     start=True, stop=True)
                nc.vector.tensor_copy(
                    hT[:, 4 * t4:4 * t4 + 4, 0:C],
                    pp[:].rearrange("p (a b) -> p a b", a=4))
            for t in range(NJ):
                nc.vector.tensor_copy(hT[:, t, C:C + 1], ones_f[:])

            # ---- main attention loop ----
            res = sb.tile([C, N], F32)    # final output staging
            groups = []
            j0 = 0
            while j0 < NJ:
                groups.append((j0, min(GROUP, NJ - j0)))
                j0 += GROUP

            def emit_o(po, e, j0, glen):
                for k in range(glen):
                    j = j0 + k
                    nc.tensor.matmul(
                        po[j % 2][:], hT[:, j, :],
                        e[:, k * ICHUNK:(k + 1) * ICHUNK],
                        start=(j < 2), stop=(j >= NJ - 2))

            def emit_fin(po, q):
                # finalize, all off the PE: merge halves, recip, DMA-bounce
                # broadcast of 1/den, divide, residual
                qs = slice(q * ICHUNK, (q + 1) * ICHUNK)
                oc = fin.tile([C + 1, ICHUNK], F32, tag="oc",
                              name=f"oc{q}")
                nc.vector.tensor_copy(oc[:], po[0][:])
                nc.vector.tensor_add(oc[:], oc[:], po[1][:])
                r = fin.tile([1, ICHUNK], F32, tag="r", name=f"r{q}")
                nc.vector.reciprocal(r[:], oc[C:C + 1, :])
                scr = dram.tile([1, ICHUNK], F32, tag="scr", name=f"scr{q}")
                nc.sync.dma_start(scr[:], r[:])
                rb = fin.tile([C, ICHUNK], F32, tag="rb", name=f"rb{q}")
                nc.sync.dma_start(rb[:], scr[:].to_broadcast((C, ICHUNK)))
                nc.vector.tensor_mul(res[:, qs], oc[0:C, :], rb[:])
                nc.vector.tensor_add(res[:, qs], res[:, qs], tin[0:C, qs])
                nc.sync.dma_start(out[:, qs], res[:, qs])

            # Software-pipelined emission: the o-matmuls of group m are
            # emitted AFTER the s-trio of group m+1, so the in-order PE
            # queue can run that s-trio while ACT is still computing
            # exp(m) (o-matmuls of m must wait for it).
            gidx = 0          # global group counter for A/B alternation
            pend_o = None     # (po, e, j0, glen) of the previous group
            pend_fin = None   # (po, q) once a chunk's last o is emitted
            po = None
            for q in range(NI):
                qs = slice(q * ICHUNK, (q + 1) * ICHUNK)
                po1 = psO1.tile([C + 1, ICHUNK], F32, tag="po1", name=f"po1_{q}")
                po2 = psO2.tile([C + 1, ICHUNK], F32, tag="po2", name=f"po2_{q}")
                po = [po1, po2]
                for gi, (j0, glen) in enumerate(groups):
                    pool = psA if gidx % 2 == 0 else psB
                    gidx += 1
                    pm = pool.tile([JT, GROUP * ICHUNK], F32, tag="m")
                    for k in range(glen):
                        j = j0 + k
                        nc.tensor.matmul(
                            pm[:, k * ICHUNK:(k + 1) * ICHUNK],
                            g_sb[32 * k:32 * k + C8, j * JT:(j + 1) * JT],
                            f_sb[32 * k:32 * k + C8, qs],
                            start=True, stop=True)
                    e = epool.tile([JT, GROUP * ICHUNK], MMDT, tag="e")
                    nc.scalar.activation(e[:, 0:glen * ICHUNK],
                                         pm[:, 0:glen * ICHUNK],
                                         mybir.ActivationFunctionType.Exp)
                    if pend_o is not None:
                        emit_o(*pend_o)
                    if pend_fin is not None:
                        emit_fin(*pend_fin)
                        pend_fin = None
                    pend_o = (po, e, j0, glen)
                pend_fin = (po, q)
            emit_o(*pend_o)
            emit_fin(*pend_fin)
    nc.compile()
    return nc


def _marshal(x_b, Wf, bf, Wg, bg, Wh, bh):
    """Build the per-core [65, 4176] input block."""
    xa = np.empty((C + 1, N + 16 + C), dtype=np.float32)
    xa[0:C, 0:N] = x_b.reshape(C, N)
    xa[C, 0:N] = 1.0
    xa[0:C, N:N + C8] = Wf.T
    xa[C, N:N + C8] = bf
    xa[0:C, N + C8:N + 16] = Wg.T
    xa[C, N + C8:N + 16] = bg
    xa[0:C, N + 16:N + 16 + C] = Wh.T
    xa[C, N + 16:N + 16 + C] = bh
    return xa


LAST_RESULTS = None


def kernel(x, Wf, bf, Wg, bg, Wh, bh):
    global LAST_RESULTS
    x = np.asarray(x, dtype=np.float32)
    Wf = np.asarray(Wf, dtype=np.float32)
    bf = np.asarray(bf, dtype=np.float32)
    Wg = np.asarray(Wg, dtype=np.float32)
    bg = np.asarray(bg, dtype=np.float32)
    Wh = np.asarray(Wh, dtype=np.float32)
    bh = np.asarray(bh, dtype=np.float32)

    if "nc" not in _CACHE:
        _CACHE["nc"] = _build_nc()
    nc = _CACHE["nc"]

    in_maps = [{"inp": _marshal(x[b], Wf, bf, Wg, bg, Wh, bh)}
               for b in range(NCORES)]
    res = run_bass_kernel_spmd(nc, in_maps, list(range(NCORES)))
    LAST_RESULTS = res
    out = np.stack([res.results[b]["out"] for b in range(NCORES)], axis=0)
    return out.reshape(B, C, H, W).astype(np.float32)
